# revision 11
# baseline (speedup 1.0000x reference)
"""Trainium2 Bass kernel v2: windowed-LSTM local attention + linear head.

Data-parallel over batch: 8 sequences -> 8 NeuronCores.  Feature-on-
partitions layout: every big tile is [128 part, 512 pos].

Software pipeline over the 3 windows (sequential recurrences, overlapped
weight DMA):

  wih(0) dma -> proj(0) -> [whh(0) dma] rec(0) { proj(1), wih(2+)/whh(1) dma }
             -> rec(1) { proj(2), whh(2) dma } -> rec(2) -> softmax+head

  - wih pool bufs=1 (36KB/part): wih(k+1) DMA waits only on proj(k) MMs
  - whh pool bufs=2 (72KB/part): whh(k+1) lands while rec(k) still runs
  - Pt (gate input projections) double-buffered per tag (48KB/part)
  - identity matmul folds P+bias into the gate PSUM accumulation
  - cell state c and the whole elementwise chain in bf16 (2x DVE rate)
  - proj bias-add on ACT (Identity+bias) to keep DVE under PE
  - logits written [9, 512] contiguous; host transposes

rep>1 wraps the whole body in For_i for slope-based device timing.
"""

import math
import numpy as np
import ml_dtypes

import concourse.bacc as bacc
import concourse.bass as bass
import concourse.tile as tile
from concourse import mybir
from concourse import bass_utils

B, L, D = 8, 512, 768
NL = 9
WINDOWS = (3, 5, 7)
NW = len(WINDOWS)
G4 = 4 * D
P = 128
ND = D // P          # 6
NM = G4 // P         # 24
N_CORES = 8

F32 = mybir.dt.float32
BF16 = mybir.dt.bfloat16
FP8 = mybir.dt.float8e4
AF = mybir.ActivationFunctionType


def _emit(tc, io, rep=1, ident_mm=True, fp8_proj=False, rec_dr=True):
    nc = tc.nc
    from contextlib import ExitStack

    with ExitStack() as ctx:
        ctx.enter_context(nc.allow_low_precision(
            reason="bf16 cell state / softmax within 2e-2 rel tolerance"))
        const = ctx.enter_context(tc.tile_pool(name="const", bufs=1))
        wih_p = ctx.enter_context(tc.tile_pool(name="wih_p", bufs=1))
        whh_p = ctx.enter_context(tc.tile_pool(name="whh_p", bufs=1))
        ppool = ctx.enter_context(tc.tile_pool(name="ppool", bufs=2))
        state = ctx.enter_context(tc.tile_pool(name="state", bufs=1))
        post = ctx.enter_context(tc.tile_pool(name="post", bufs=8))
        tmp = ctx.enter_context(tc.tile_pool(name="tmp", bufs=6))
        attn = ctx.enter_context(tc.tile_pool(name="attn", bufs=1))
        psum = ctx.enter_context(tc.tile_pool(name="psum", bufs=8, space="PSUM"))

        # ---- constants resident in SBUF (outside the rep loop) ----
        xb = []   # x.T bf16: proj rhs, attention dot, head residual
        for dc in range(ND):
            t_b = const.tile([P, L], BF16, tag=f"xb{dc}", name=f"xb{dc}")
            nc.sync.dma_start(t_b, io["xb"].ap()[dc * P:(dc + 1) * P, :])
            xb.append(t_b)

        x8 = const.tile([P, ND, L], FP8, tag="x8")
        nc.sync.dma_start(x8, io["x8"].ap().rearrange("p (n l) -> p n l", n=ND))

        # bias pre-laid-out on host as [P, NW*NM]: col k*NM+m = bias[k, m*128+p]
        bias_sb = const.tile([P, NW * NM], F32, tag="bias")
        nc.sync.dma_start(bias_sb, io["bias"].ap())

        lw = []
        for dc in range(ND):
            t = const.tile([P, NL], BF16, tag=f"lw{dc}", name=f"lw{dc}")
            nc.sync.dma_start(t, io["lwt"].ap()[dc * P:(dc + 1) * P, :])
            lw.append(t)
        lb_sb = const.tile([NL, 1], F32, tag="lb")
        nc.sync.dma_start(lb_sb, io["lb"].ap().rearrange("(c o) -> c o", o=1))

        ident_sb = const.tile([P, P], BF16, tag="ident")
        nc.sync.dma_start(ident_sb, io["ident"].ap())

        ones_mat = const.tile([P, P], BF16, tag="ones_mat")
        nc.vector.memset(ones_mat, 1.0)

        # persistent state tiles (written fully each rep iteration)
        c = [state.tile([P, L], BF16, tag=f"c{dc}", name=f"c{dc}")
             for dc in range(ND)]
        h8 = state.tile([P, ND, L], FP8, tag="h8")
        hks = [[state.tile([P, L], BF16, tag=f"h{k}_{dc}", name=f"h{k}_{dc}")
                for dc in range(ND)] for k in range(NW)]
        ak_t = [state.tile([P, L], BF16, tag=f"ak{k}", name=f"akt{k}")
                for k in range(NW)]
        logits = const.tile([NL, L], F32, tag="logits")

        inv_sqrt_d = 1.0 / math.sqrt(D)

        def load_wih(k):
            t = wih_p.tile([P, ND, G4], FP8, tag="A8", name=f"A8_{k}")
            nc.sync.dma_start(
                t, io["wih"].ap()[k].rearrange("p (n g) -> p n g", n=ND))
            return t

        def load_whh(k):
            t = whh_p.tile([P, ND, G4], FP8, tag="B8", name=f"B8_{k}", bufs=2)
            nc.sync.dma_start(
                t, io["whh"].ap()[k].rearrange("p (n g) -> p n g", n=ND))
            return t

        PROJ_ORDER = (list(range(0, 6)) + list(range(12, 24))
                      + list(range(6, 12)))

        def proj(k, wih):
            """Pt[k] = bias + Wih(k) @ x; t0-needed gates (i,g,o) first."""
            Ptd = {}
            for m in PROJ_ORDER:
                ps = psum.tile([P, L], F32, tag="g", name=f"pj{k}_{m}", bufs=8)
                if fp8_proj:
                    for j in range(0, ND, 2):
                        nc.tensor.matmul(
                            ps,
                            lhsT=wih[:, j:j + 2, m * P:(m + 1) * P],
                            rhs=x8[:, j:j + 2, :],
                            start=(j == 0),
                            stop=(j == ND - 2),
                            perf_mode=mybir.MatmulPerfMode.DoubleRow,
                        )
                else:
                    for j in range(ND):
                        nc.tensor.matmul(
                            ps,
                            lhsT=wih[:, j, m * P:(m + 1) * P],
                            rhs=x8[:, j, :],
                            start=(j == 0),
                            stop=(j == ND - 1),
                        )
                pt = ppool.tile([P, L], BF16, tag=f"P{m}", name=f"P{k}_{m}",
                                bufs=2)
                nc.vector.tensor_scalar_add(
                    pt, ps, bias_sb[:, k * NM + m:k * NM + m + 1])
                Ptd[m] = pt
            return [Ptd[m] for m in range(NM)]

        def rec_step(k, w, t, whh, h, Ptk):
            hw_ = w // 2
            off = t - hw_
            s = max(0, -off)
            e = min(L, L - off)
            n = e - s
            last = (t == w - 1)

            def h_out(dc):
                # intermediate steps feed the fp8 DoubleRow matmuls; the
                # final step lands in bf16 locals for attention/head
                return h[dc][:, s:e] if last else h8[:, dc, s:e]

            if t == 0:
                for dc in range(ND):
                    i_t = post.tile([P, L], BF16, tag="post", name="i0", bufs=8)
                    nc.scalar.activation(
                        i_t[:, :n], Ptk[0 + dc][:, s + off:e + off], AF.Sigmoid)
                    g_t = post.tile([P, L], BF16, tag="post", name="g0", bufs=8)
                    nc.scalar.activation(
                        g_t[:, :n], Ptk[12 + dc][:, s + off:e + off], AF.Tanh)
                    o_t = post.tile([P, L], BF16, tag="post", name="o0", bufs=8)
                    nc.scalar.activation(
                        o_t[:, :n], Ptk[18 + dc][:, s + off:e + off], AF.Sigmoid)
                    if s > 0:
                        nc.vector.memset(c[dc][:, 0:s], 0.0)
                        nc.vector.memset(h8[:, dc, 0:s], 0.0)
                    nc.vector.tensor_mul(c[dc][:, s:e], i_t[:, :n], g_t[:, :n])
                    tc_t = post.tile([P, L], BF16, tag="post", name="tc0", bufs=8)
                    nc.scalar.activation(tc_t[:, :n], c[dc][:, s:e], AF.Tanh)
                    nc.vector.tensor_mul(h8[:, dc, s:e], o_t[:, :n], tc_t[:, :n])
                return

            for dc in range(ND):
                gp = {}
                for base in (0, 12, 6, 18):
                    m = base + dc
                    ps = psum.tile([P, L], F32, tag="g", name=f"s{t}_{m}", bufs=8)
                    nc.tensor.matmul(
                        ps[:, s:e],
                        lhsT=ident_sb[:],
                        rhs=Ptk[m][:, s + off:e + off],
                        start=True,
                        stop=False,
                    )
                    if rec_dr:
                        for j in range(0, ND, 2):
                            nc.tensor.matmul(
                                ps[:, s:e],
                                lhsT=whh[:, j:j + 2, m * P:(m + 1) * P],
                                rhs=h8[:, j:j + 2, s:e],
                                start=False,
                                stop=(j == ND - 2),
                                perf_mode=mybir.MatmulPerfMode.DoubleRow,
                            )
                    else:
                        for j in range(ND):
                            nc.tensor.matmul(
                                ps[:, s:e],
                                lhsT=whh[:, j, m * P:(m + 1) * P],
                                rhs=h8[:, j, s:e],
                                start=False,
                                stop=(j == ND - 1),
                            )
                    gp[base] = ps

                acts = {}
                for base, fn in ((0, AF.Sigmoid), (12, AF.Tanh),
                                 (6, AF.Sigmoid), (18, AF.Sigmoid)):
                    a = post.tile([P, L], BF16, tag="post", name=f"a{base}",
                                  bufs=8)
                    nc.scalar.activation(a[:, :n], gp[base][:, s:e], fn)
                    acts[base] = a
                i_t, f_t, g_t, o_t = acts[0], acts[6], acts[12], acts[18]

                t1 = tmp.tile([P, L], BF16, tag="tmp", name="t1", bufs=6)
                nc.vector.tensor_mul(t1[:, :n], i_t[:, :n], g_t[:, :n])
                t2 = tmp.tile([P, L], BF16, tag="tmp", name="t2", bufs=6)
                nc.vector.tensor_mul(t2[:, :n], f_t[:, :n], c[dc][:, s:e])
                nc.vector.tensor_add(c[dc][:, s:e], t1[:, :n], t2[:, :n])
                tc_t = post.tile([P, L], BF16, tag="post", name="tct", bufs=8)
                nc.scalar.activation(tc_t[:, :n], c[dc][:, s:e], AF.Tanh)
                nc.vector.tensor_mul(h_out(dc), o_t[:, :n], tc_t[:, :n])
                if last and e < L:
                    nc.vector.tensor_copy(h[dc][:, e:L], h8[:, dc, e:L])

        def attn_dot(k, h):
            # broadcast dot: ones[P,P].T @ td accumulates the full x.h dot
            # into EVERY partition, so the 3-way softmax runs as [P, L] ops
            psd = psum.tile([P, L], F32, tag="g", name=f"dot{k}", bufs=8)
            for dc in range(ND):
                td = tmp.tile([P, L], BF16, tag="tmp", name="td", bufs=6)
                nc.vector.tensor_mul(td, xb[dc][:], h[dc][:])
                nc.tensor.matmul(
                    psd,
                    lhsT=ones_mat[:],
                    rhs=td[:],
                    start=(dc == 0),
                    stop=(dc == ND - 1),
                )
            nc.scalar.activation(ak_t[k], psd, AF.Copy, scale=inv_sqrt_d)

        def body():
            wih = load_wih(0)
            Ptk = [None] * NW
            Ptk[0] = proj(0, wih)
            whh_cur = load_whh(0)

            for k, w in enumerate(WINDOWS):
                h = hks[k]
                for t in range(w):
                    rec_step(k, w, t, whh_cur, h, Ptk[k])
                    if t == 1 and k + 1 < NW:
                        # overlap: next window's input proj + weight loads
                        wih2 = load_wih(k + 1)
                        Ptk[k + 1] = proj(k + 1, wih2)
                        whh_nxt = load_whh(k + 1)
                attn_dot(k, h)
                if k + 1 < NW:
                    whh_cur = whh_nxt

            # ===== softmax over 3 window outputs (broadcast [P, L] form) =====
            mx1 = tmp.tile([P, L], BF16, tag="tmp", name="mx1", bufs=6)
            nc.vector.tensor_max(mx1, ak_t[0][:], ak_t[1][:])
            mx2 = tmp.tile([P, L], BF16, tag="tmp", name="mx2", bufs=6)
            nc.vector.tensor_max(mx2, mx1[:], ak_t[2][:])
            e_sb = []
            for k in range(NW):
                d_k = tmp.tile([P, L], BF16, tag="tmp", name=f"dk{k}", bufs=6)
                nc.vector.tensor_sub(d_k, ak_t[k][:], mx2[:])
                ek = attn.tile([P, L], BF16, tag=f"ek{k}", name=f"ek{k}")
                nc.scalar.activation(ek, d_k, AF.Exp)
                e_sb.append(ek)
            s1 = tmp.tile([P, L], BF16, tag="tmp", name="s1", bufs=6)
            nc.vector.tensor_add(s1, e_sb[0][:], e_sb[1][:])
            s2 = tmp.tile([P, L], BF16, tag="tmp", name="s2", bufs=6)
            nc.vector.tensor_add(s2, s1[:], e_sb[2][:])
            r = attn.tile([P, L], BF16, tag="rr", name="rr")
            nc.vector.reciprocal(r, s2[:])


            ps_log = psum.tile([NL, L], F32, tag="g", name="pslog", bufs=8)
            for dc in range(ND):
                lf = tmp.tile([P, L], BF16, tag="tmp", name="lf", bufs=6)
                nc.vector.tensor_mul(lf, e_sb[0][:], hks[0][dc][:])
                t3 = tmp.tile([P, L], BF16, tag="tmp", name="t3", bufs=6)
                nc.vector.tensor_mul(t3, e_sb[1][:], hks[1][dc][:])
                lf2 = tmp.tile([P, L], BF16, tag="tmp", name="lf2", bufs=6)
                nc.vector.tensor_add(lf2, lf[:], t3[:])
                t4 = tmp.tile([P, L], BF16, tag="tmp", name="t4", bufs=6)
                nc.vector.tensor_mul(t4, e_sb[2][:], hks[2][dc][:])
                lf3 = tmp.tile([P, L], BF16, tag="tmp", name="lf3", bufs=6)
                nc.vector.tensor_add(lf3, lf2[:], t4[:])
                lf4 = tmp.tile([P, L], BF16, tag="tmp", name="lf4", bufs=6)
                nc.vector.tensor_mul(lf4, lf3[:], r[:])
                feat = tmp.tile([P, L], BF16, tag="tmp", name="feat", bufs=6)
                nc.vector.tensor_add(feat, lf4[:], xb[dc][:])
                nc.tensor.matmul(ps_log, lhsT=lw[dc][:], rhs=feat[:],
                                 start=(dc == 0), stop=(dc == ND - 1))
            nc.scalar.activation(logits, ps_log, AF.Identity, bias=lb_sb[:, 0:1])
            nc.sync.dma_start(io["out"].ap(), logits[:])

        if rep == 1:
            body()
        else:
            with tc.For_i(0, rep, 1) as _i:
                body()


_NC_CACHE = {}


def _get_nc(rep=1, ident_mm=True, fp8_proj=False, rec_dr=True):
    key = (rep, ident_mm, fp8_proj, rec_dr)
    if key not in _NC_CACHE:
        nc = bacc.Bacc("TRN2", target_bir_lowering=False, debug=False)
        io = {
            "xb": nc.dram_tensor("xb", [D, L], BF16, kind="ExternalInput"),
            "x8": nc.dram_tensor("x8", [P, ND * L], FP8, kind="ExternalInput"),
            "wih": nc.dram_tensor("wih", [NW, P, ND * G4], FP8, kind="ExternalInput"),
            "whh": nc.dram_tensor("whh", [NW, P, ND * G4], FP8, kind="ExternalInput"),
            "bias": nc.dram_tensor("bias", [P, NW * NM], F32, kind="ExternalInput"),
            "lwt": nc.dram_tensor("lwt", [D, NL], BF16, kind="ExternalInput"),
            "lb": nc.dram_tensor("lb", [NL], F32, kind="ExternalInput"),
            "ident": nc.dram_tensor("ident", [P, P], BF16, kind="ExternalInput"),
            "out": nc.dram_tensor("out", [NL, L], F32, kind="ExternalOutput"),
        }
        with tile.TileContext(nc) as tc:
            _emit(tc, io, rep=rep, ident_mm=ident_mm, fp8_proj=fp8_proj, rec_dr=rec_dr)
        nc.compile()
        _NC_CACHE[key] = nc
    return _NC_CACHE[key]


def _in_maps(sequence_output, W_ih, W_hh, b_ih, b_hh, lin_w, lin_b):
    x = np.asarray(sequence_output, np.float32)
    wih_f = np.transpose(np.asarray(W_ih, np.float32), (0, 2, 1))  # [NW, D, G4]
    Wih8 = np.ascontiguousarray(
        wih_f.reshape(NW, ND, P, G4).transpose(0, 2, 1, 3).reshape(NW, P, ND * G4)
    ).astype(ml_dtypes.float8_e4m3fn)
    whh_f = np.transpose(np.asarray(W_hh, np.float32), (0, 2, 1))  # [NW, D, G4]
    Whh8 = np.ascontiguousarray(
        whh_f.reshape(NW, ND, P, G4).transpose(0, 2, 1, 3).reshape(NW, P, ND * G4)
    ).astype(ml_dtypes.float8_e4m3fn)
    biasc = np.asarray(b_ih, np.float32) + np.asarray(b_hh, np.float32)
    bias_pm = np.ascontiguousarray(
        biasc.reshape(NW, NM, P).transpose(2, 0, 1).reshape(P, NW * NM))
    lwt = np.ascontiguousarray(
        np.asarray(lin_w, np.float32).T).astype(ml_dtypes.bfloat16)
    lb = np.asarray(lin_b, np.float32)
    ident = np.eye(P, dtype=np.float32).astype(ml_dtypes.bfloat16)
    maps = []
    for b in range(B):
        xT = np.ascontiguousarray(x[b].T)
        x8 = np.ascontiguousarray(
            xT.reshape(ND, P, L).transpose(1, 0, 2).reshape(P, ND * L)
        ).astype(ml_dtypes.float8_e4m3fn)
        maps.append({
            "xb": xT.astype(ml_dtypes.bfloat16),
            "x8": x8,
            "wih": Wih8,
            "whh": Whh8,
            "bias": bias_pm,
            "lwt": lwt,
            "lb": lb,
            "ident": ident,
        })
    return maps


def kernel(sequence_output, W_ih, W_hh, b_ih, b_hh, lin_w, lin_b):
    nc = _get_nc()
    maps = _in_maps(sequence_output, W_ih, W_hh, b_ih, b_hh, lin_w, lin_b)
    res = bass_utils.run_bass_kernel_spmd(nc, maps, core_ids=list(range(N_CORES)))
    return np.stack(
        [np.ascontiguousarray(res.results[b]["out"].T) for b in range(B)], axis=0)


# revision 12
# speedup vs baseline: 1.5155x; 1.5155x over previous
"""Trainium2 Bass kernel v2: windowed-LSTM local attention + linear head.

Data-parallel over batch: 8 sequences -> 8 NeuronCores.  Feature-on-
partitions layout: every big tile is [128 part, 512 pos].

Software pipeline over the 3 windows (sequential recurrences, overlapped
weight DMA):

  wih(0) dma -> proj(0) -> [whh(0) dma] rec(0) { proj(1), wih(2+)/whh(1) dma }
             -> rec(1) { proj(2), whh(2) dma } -> rec(2) -> softmax+head

  - wih pool bufs=1 (36KB/part): wih(k+1) DMA waits only on proj(k) MMs
  - whh pool bufs=2 (72KB/part): whh(k+1) lands while rec(k) still runs
  - Pt (gate input projections) double-buffered per tag (48KB/part)
  - identity matmul folds P+bias into the gate PSUM accumulation
  - cell state c and the whole elementwise chain in bf16 (2x DVE rate)
  - proj bias-add on ACT (Identity+bias) to keep DVE under PE
  - logits written [9, 512] contiguous; host transposes

rep>1 wraps the whole body in For_i for slope-based device timing.
"""

import math
import numpy as np
import ml_dtypes

import concourse.bacc as bacc
import concourse.bass as bass
import concourse.tile as tile
from concourse import mybir
from concourse import bass_utils

B, L, D = 8, 512, 768
NL = 9
WINDOWS = (3, 5, 7)
NW = len(WINDOWS)
G4 = 4 * D
P = 128
ND = D // P          # 6
NM = G4 // P         # 24
N_CORES = 8

F32 = mybir.dt.float32
BF16 = mybir.dt.bfloat16
FP8 = mybir.dt.float8e4
AF = mybir.ActivationFunctionType


def _emit(tc, io, rep=1, ident_mm=True, fp8_proj=False, rec_dr=True):
    nc = tc.nc
    from contextlib import ExitStack

    with ExitStack() as ctx:
        ctx.enter_context(nc.allow_low_precision(
            reason="bf16 cell state / softmax within 2e-2 rel tolerance"))
        const = ctx.enter_context(tc.tile_pool(name="const", bufs=1))
        wih_p = ctx.enter_context(tc.tile_pool(name="wih_p", bufs=1))
        whh_p = ctx.enter_context(tc.tile_pool(name="whh_p", bufs=1))
        ppool = ctx.enter_context(tc.tile_pool(name="ppool", bufs=2))
        state = ctx.enter_context(tc.tile_pool(name="state", bufs=1))
        post = ctx.enter_context(tc.tile_pool(name="post", bufs=8))
        tmp = ctx.enter_context(tc.tile_pool(name="tmp", bufs=6))
        attn = ctx.enter_context(tc.tile_pool(name="attn", bufs=1))
        psum = ctx.enter_context(tc.tile_pool(name="psum", bufs=8, space="PSUM"))

        # ---- constants resident in SBUF (outside the rep loop) ----
        xb = []   # x.T bf16: proj rhs, attention dot, head residual
        for dc in range(ND):
            t_b = const.tile([P, L], BF16, tag=f"xb{dc}", name=f"xb{dc}")
            nc.sync.dma_start(t_b, io["xb"].ap()[dc * P:(dc + 1) * P, :])
            xb.append(t_b)

        x8 = const.tile([P, ND, L], FP8, tag="x8")
        nc.sync.dma_start(x8, io["x8"].ap().rearrange("p (n l) -> p n l", n=ND))

        # bias pre-laid-out on host as [P, NW*NM]: col k*NM+m = bias[k, m*128+p]
        bias_sb = const.tile([P, NW * NM], F32, tag="bias")
        nc.sync.dma_start(bias_sb, io["bias"].ap())

        lw = []
        for dc in range(ND):
            t = const.tile([P, NL], BF16, tag=f"lw{dc}", name=f"lw{dc}")
            nc.sync.dma_start(t, io["lwt"].ap()[dc * P:(dc + 1) * P, :])
            lw.append(t)
        lb_sb = const.tile([NL, 1], F32, tag="lb")
        nc.sync.dma_start(lb_sb, io["lb"].ap().rearrange("(c o) -> c o", o=1))

        ident_sb = const.tile([P, P], BF16, tag="ident")
        nc.sync.dma_start(ident_sb, io["ident"].ap())

        ones_mat = const.tile([P, P], BF16, tag="ones_mat")
        nc.vector.memset(ones_mat, 1.0)

        # persistent state tiles (written fully each rep iteration)
        c = [state.tile([P, L], BF16, tag=f"c{dc}", name=f"c{dc}")
             for dc in range(ND)]
        h8 = state.tile([P, ND, L], FP8, tag="h8")
        hks = [[state.tile([P, L], BF16, tag=f"h{k}_{dc}", name=f"h{k}_{dc}")
                for dc in range(ND)] for k in range(NW)]
        ak_t = [state.tile([P, L], BF16, tag=f"ak{k}", name=f"akt{k}")
                for k in range(NW)]
        logits = const.tile([NL, L], F32, tag="logits")

        inv_sqrt_d = 1.0 / math.sqrt(D)

        def load_wih(k):
            t = wih_p.tile([P, ND, G4], FP8, tag="A8", name=f"A8_{k}")
            nc.sync.dma_start(
                t, io["wih"].ap()[k].rearrange("p (n g) -> p n g", n=ND))
            return t

        def load_whh(k):
            t = whh_p.tile([P, ND, G4], FP8, tag="B8", name=f"B8_{k}", bufs=2)
            nc.sync.dma_start(
                t, io["whh"].ap()[k].rearrange("p (n g) -> p n g", n=ND))
            return t

        PROJ_ORDER = (list(range(0, 6)) + list(range(12, 24))
                      + list(range(6, 12)))

        def proj(k, wih):
            """Pt[k] = bias + Wih(k) @ x; t0-needed gates (i,g,o) first."""
            Ptd = {}
            for m in PROJ_ORDER:
                ps = psum.tile([P, L], F32, tag="g", name=f"pj{k}_{m}", bufs=8)
                if fp8_proj:
                    for j in range(0, ND, 2):
                        nc.tensor.matmul(
                            ps,
                            lhsT=wih[:, j:j + 2, m * P:(m + 1) * P],
                            rhs=x8[:, j:j + 2, :],
                            start=(j == 0),
                            stop=(j == ND - 2),
                            perf_mode=mybir.MatmulPerfMode.DoubleRow,
                        )
                else:
                    for j in range(ND):
                        nc.tensor.matmul(
                            ps,
                            lhsT=wih[:, j, m * P:(m + 1) * P],
                            rhs=x8[:, j, :],
                            start=(j == 0),
                            stop=(j == ND - 1),
                        )
                pt = ppool.tile([P, L], BF16, tag=f"P{m}", name=f"P{k}_{m}",
                                bufs=2)
                nc.vector.tensor_scalar_add(
                    pt, ps, bias_sb[:, k * NM + m:k * NM + m + 1])
                Ptd[m] = pt
            return [Ptd[m] for m in range(NM)]

        def rec_step(k, w, t, whh, h, Ptk, c, h8):
            hw_ = w // 2
            off = t - hw_
            s = max(0, -off)
            e = min(L, L - off)
            n = e - s
            last = (t == w - 1)

            def h_out(dc):
                # intermediate steps feed the fp8 DoubleRow matmuls; the
                # final step lands in bf16 locals for attention/head
                return h[dc][:, s:e] if last else h8[:, dc, s:e]

            if t == 0:
                for dc in range(ND):
                    i_t = post.tile([P, L], BF16, tag="post", name="i0", bufs=8)
                    nc.scalar.activation(
                        i_t[:, :n], Ptk[0 + dc][:, s + off:e + off], AF.Sigmoid)
                    g_t = post.tile([P, L], BF16, tag="post", name="g0", bufs=8)
                    nc.scalar.activation(
                        g_t[:, :n], Ptk[12 + dc][:, s + off:e + off], AF.Tanh)
                    o_t = post.tile([P, L], BF16, tag="post", name="o0", bufs=8)
                    nc.scalar.activation(
                        o_t[:, :n], Ptk[18 + dc][:, s + off:e + off], AF.Sigmoid)
                    if s > 0:
                        nc.vector.memset(c[dc][:, 0:s], 0.0)
                        nc.vector.memset(h8[:, dc, 0:s], 0.0)
                    nc.vector.tensor_mul(c[dc][:, s:e], i_t[:, :n], g_t[:, :n])
                    tc_t = post.tile([P, L], BF16, tag="post", name="tc0", bufs=8)
                    nc.scalar.activation(tc_t[:, :n], c[dc][:, s:e], AF.Tanh)
                    nc.vector.tensor_mul(h8[:, dc, s:e], o_t[:, :n], tc_t[:, :n])
                return

            for dc in range(ND):
                gp = {}
                for base in (0, 12, 6, 18):
                    m = base + dc
                    ps = psum.tile([P, L], F32, tag="g", name=f"s{t}_{m}", bufs=8)
                    nc.tensor.matmul(
                        ps[:, s:e],
                        lhsT=ident_sb[:],
                        rhs=Ptk[m][:, s + off:e + off],
                        start=True,
                        stop=False,
                    )
                    if rec_dr:
                        for j in range(0, ND, 2):
                            nc.tensor.matmul(
                                ps[:, s:e],
                                lhsT=whh[:, j:j + 2, m * P:(m + 1) * P],
                                rhs=h8[:, j:j + 2, s:e],
                                start=False,
                                stop=(j == ND - 2),
                                perf_mode=mybir.MatmulPerfMode.DoubleRow,
                            )
                    else:
                        for j in range(ND):
                            nc.tensor.matmul(
                                ps[:, s:e],
                                lhsT=whh[:, j, m * P:(m + 1) * P],
                                rhs=h8[:, j, s:e],
                                start=False,
                                stop=(j == ND - 1),
                            )
                    gp[base] = ps

                acts = {}
                for base, fn in ((0, AF.Sigmoid), (12, AF.Tanh),
                                 (6, AF.Sigmoid), (18, AF.Sigmoid)):
                    a = post.tile([P, L], BF16, tag="post", name=f"a{base}",
                                  bufs=8)
                    nc.scalar.activation(a[:, :n], gp[base][:, s:e], fn)
                    acts[base] = a
                i_t, f_t, g_t, o_t = acts[0], acts[6], acts[12], acts[18]

                t1 = tmp.tile([P, L], BF16, tag="tmp", name="t1", bufs=6)
                nc.vector.tensor_mul(t1[:, :n], i_t[:, :n], g_t[:, :n])
                t2 = tmp.tile([P, L], BF16, tag="tmp", name="t2", bufs=6)
                nc.vector.tensor_mul(t2[:, :n], f_t[:, :n], c[dc][:, s:e])
                nc.vector.tensor_add(c[dc][:, s:e], t1[:, :n], t2[:, :n])
                tc_t = post.tile([P, L], BF16, tag="post", name="tct", bufs=8)
                nc.scalar.activation(tc_t[:, :n], c[dc][:, s:e], AF.Tanh)
                nc.vector.tensor_mul(h_out(dc), o_t[:, :n], tc_t[:, :n])
                if last and e < L:
                    nc.vector.tensor_copy(h[dc][:, e:L], h8[:, dc, e:L])

        def attn_dot(k, h):
            # broadcast dot: ones[P,P].T @ td accumulates the full x.h dot
            # into EVERY partition, so the 3-way softmax runs as [P, L] ops
            psd = psum.tile([P, L], F32, tag="g", name=f"dot{k}", bufs=8)
            for dc in range(ND):
                td = tmp.tile([P, L], BF16, tag="tmp", name="td", bufs=6)
                nc.vector.tensor_mul(td, xb[dc][:], h[dc][:])
                nc.tensor.matmul(
                    psd,
                    lhsT=ones_mat[:],
                    rhs=td[:],
                    start=(dc == 0),
                    stop=(dc == ND - 1),
                )
            nc.scalar.activation(ak_t[k], psd, AF.Copy, scale=inv_sqrt_d)

        def body():
            wih = load_wih(0)
            Ptk = [None] * NW
            Ptk[0] = proj(0, wih)
            whh_cur = load_whh(0)

            for k, w in enumerate(WINDOWS):
                h = hks[k]
                for t in range(w):
                    rec_step(k, w, t, whh_cur, h, Ptk[k])
                    if t == 1 and k + 1 < NW:
                        # overlap: next window's input proj + weight loads
                        wih2 = load_wih(k + 1)
                        Ptk[k + 1] = proj(k + 1, wih2)
                        whh_nxt = load_whh(k + 1)
                attn_dot(k, h)
                if k + 1 < NW:
                    whh_cur = whh_nxt

            # ===== softmax over 3 window outputs (broadcast [P, L] form) =====
            mx1 = tmp.tile([P, L], BF16, tag="tmp", name="mx1", bufs=6)
            nc.vector.tensor_max(mx1, ak_t[0][:], ak_t[1][:])
            mx2 = tmp.tile([P, L], BF16, tag="tmp", name="mx2", bufs=6)
            nc.vector.tensor_max(mx2, mx1[:], ak_t[2][:])
            e_sb = []
            for k in range(NW):
                d_k = tmp.tile([P, L], BF16, tag="tmp", name=f"dk{k}", bufs=6)
                nc.vector.tensor_sub(d_k, ak_t[k][:], mx2[:])
                ek = attn.tile([P, L], BF16, tag=f"ek{k}", name=f"ek{k}")
                nc.scalar.activation(ek, d_k, AF.Exp)
                e_sb.append(ek)
            s1 = tmp.tile([P, L], BF16, tag="tmp", name="s1", bufs=6)
            nc.vector.tensor_add(s1, e_sb[0][:], e_sb[1][:])
            s2 = tmp.tile([P, L], BF16, tag="tmp", name="s2", bufs=6)
            nc.vector.tensor_add(s2, s1[:], e_sb[2][:])
            r = attn.tile([P, L], BF16, tag="rr", name="rr")
            nc.vector.reciprocal(r, s2[:])


            ps_log = psum.tile([NL, L], F32, tag="g", name="pslog", bufs=8)
            for dc in range(ND):
                lf = tmp.tile([P, L], BF16, tag="tmp", name="lf", bufs=6)
                nc.vector.tensor_mul(lf, e_sb[0][:], hks[0][dc][:])
                t3 = tmp.tile([P, L], BF16, tag="tmp", name="t3", bufs=6)
                nc.vector.tensor_mul(t3, e_sb[1][:], hks[1][dc][:])
                lf2 = tmp.tile([P, L], BF16, tag="tmp", name="lf2", bufs=6)
                nc.vector.tensor_add(lf2, lf[:], t3[:])
                t4 = tmp.tile([P, L], BF16, tag="tmp", name="t4", bufs=6)
                nc.vector.tensor_mul(t4, e_sb[2][:], hks[2][dc][:])
                lf3 = tmp.tile([P, L], BF16, tag="tmp", name="lf3", bufs=6)
                nc.vector.tensor_add(lf3, lf2[:], t4[:])
                lf4 = tmp.tile([P, L], BF16, tag="tmp", name="lf4", bufs=6)
                nc.vector.tensor_mul(lf4, lf3[:], r[:])
                feat = tmp.tile([P, L], BF16, tag="tmp", name="feat", bufs=6)
                nc.vector.tensor_add(feat, lf4[:], xb[dc][:])
                nc.tensor.matmul(ps_log, lhsT=lw[dc][:], rhs=feat[:],
                                 start=(dc == 0), stop=(dc == ND - 1))
            nc.scalar.activation(logits, ps_log, AF.Identity, bias=lb_sb[:, 0:1])
            nc.sync.dma_start(io["out"].ap(), logits[:])

        if rep == 1:
            body()
        else:
            with tc.For_i(0, rep, 1) as _i:
                body()


_NC_CACHE = {}


def _get_nc(rep=1, ident_mm=True, fp8_proj=False, rec_dr=True):
    key = (rep, ident_mm, fp8_proj, rec_dr)
    if key not in _NC_CACHE:
        nc = bacc.Bacc("TRN2", target_bir_lowering=False, debug=False)
        io = {
            "xb": nc.dram_tensor("xb", [D, L], BF16, kind="ExternalInput"),
            "x8": nc.dram_tensor("x8", [P, ND * L], FP8, kind="ExternalInput"),
            "wih": nc.dram_tensor("wih", [NW, P, ND * G4], FP8, kind="ExternalInput"),
            "whh": nc.dram_tensor("whh", [NW, P, ND * G4], FP8, kind="ExternalInput"),
            "bias": nc.dram_tensor("bias", [P, NW * NM], F32, kind="ExternalInput"),
            "lwt": nc.dram_tensor("lwt", [D, NL], BF16, kind="ExternalInput"),
            "lb": nc.dram_tensor("lb", [NL], F32, kind="ExternalInput"),
            "ident": nc.dram_tensor("ident", [P, P], BF16, kind="ExternalInput"),
            "out": nc.dram_tensor("out", [NL, L], F32, kind="ExternalOutput"),
        }
        with tile.TileContext(nc) as tc:
            _emit(tc, io, rep=rep, ident_mm=ident_mm, fp8_proj=fp8_proj, rec_dr=rec_dr)
        nc.compile()
        _NC_CACHE[key] = nc
    return _NC_CACHE[key]


def _in_maps(sequence_output, W_ih, W_hh, b_ih, b_hh, lin_w, lin_b):
    x = np.asarray(sequence_output, np.float32)
    wih_f = np.transpose(np.asarray(W_ih, np.float32), (0, 2, 1))  # [NW, D, G4]
    Wih8 = np.ascontiguousarray(
        wih_f.reshape(NW, ND, P, G4).transpose(0, 2, 1, 3).reshape(NW, P, ND * G4)
    ).astype(ml_dtypes.float8_e4m3fn)
    whh_f = np.transpose(np.asarray(W_hh, np.float32), (0, 2, 1))  # [NW, D, G4]
    Whh8 = np.ascontiguousarray(
        whh_f.reshape(NW, ND, P, G4).transpose(0, 2, 1, 3).reshape(NW, P, ND * G4)
    ).astype(ml_dtypes.float8_e4m3fn)
    biasc = np.asarray(b_ih, np.float32) + np.asarray(b_hh, np.float32)
    bias_pm = np.ascontiguousarray(
        biasc.reshape(NW, NM, P).transpose(2, 0, 1).reshape(P, NW * NM))
    lwt = np.ascontiguousarray(
        np.asarray(lin_w, np.float32).T).astype(ml_dtypes.bfloat16)
    lb = np.asarray(lin_b, np.float32)
    ident = np.eye(P, dtype=np.float32).astype(ml_dtypes.bfloat16)
    maps = []
    for b in range(B):
        xT = np.ascontiguousarray(x[b].T)
        x8 = np.ascontiguousarray(
            xT.reshape(ND, P, L).transpose(1, 0, 2).reshape(P, ND * L)
        ).astype(ml_dtypes.float8_e4m3fn)
        maps.append({
            "xb": xT.astype(ml_dtypes.bfloat16),
            "x8": x8,
            "wih": Wih8,
            "whh": Whh8,
            "bias": bias_pm,
            "lwt": lwt,
            "lb": lb,
            "ident": ident,
        })
    return maps


def kernel(sequence_output, W_ih, W_hh, b_ih, b_hh, lin_w, lin_b):
    nc = _get_nc()
    maps = _in_maps(sequence_output, W_ih, W_hh, b_ih, b_hh, lin_w, lin_b)
    res = bass_utils.run_bass_kernel_spmd(nc, maps, core_ids=list(range(N_CORES)))
    return np.stack(
        [np.ascontiguousarray(res.results[b]["out"].T) for b in range(B)], axis=0)
